# revision 1
# baseline (speedup 1.0000x reference)
"""Trainium2 Bass kernel for nn_Attention_79121887527485.

Multi-head causal attention with ALiBi, B=2 S=2048 D=2048 H=16 DH=128.
Tensor-parallel over heads across 8 NeuronCores: core c owns heads
c, c+8 (rows of Wq/Wk/Wv, cols of Wo). Each core computes a full
[BS, D] partial of the output projection; the host sums the 8 partials
(the unshard step for the input-sharded Wo).

Per-core device kernel (all matmuls bf16 with fp32 PSUM accumulation):
  1. QKV: Q^T, K^T produced in [dh, s] layout, V in [s, dh] layout,
     directly from x^T tiles streamed from DRAM.
  2. Attention per (head, batch, 512-wide q-chunk), causally skipping
     k-tiles above the diagonal and ALiBi-negligible ones below:
       scores^T[k, q] = (K^T tile).T @ (Q^T chunk)      (PE)
       [slot0 only] += causal/qrow mask tile            (DVE)
       [slot1 diag] += causal mask tile                 (DVE)
       P^T = exp(scale*scores^T + bias[k])              (ACT)
       z^T   += (V tile).T @ P^T                        (PE)
       l: qc<=1 chunks accumulate ones128.T @ P^T per tile on the PE;
          qc>=2 chunks accumulate ptsum += P^T on DVE (fp32 SBUF) and
          run one ones-matmul on the bf16-cast sum (saves PE streaming)
     z_norm^T = z^T * reciprocal(l) -> SBUF bf16.       (DVE)
     Softmax shift convention: slot0 (heads c) uses the exact per-query
     shift -slope*q (qrow/mask adds); slot1 (heads c+8, small slopes)
     uses a per-chunk constant shift slope*(k - chunk_end) - C0, valid
     because a per-query scale factor cancels in z/l, and the argument
     spread slope*511 <= 23 fits fp32/bf16 comfortably.
  3. Output projection per (b, s-chunk): out^T[o, s] = Wo_c^T.T @ z^T,
     written as fp16, DMAed per [128, 512] tile.
"""

import math
from contextlib import ExitStack

import numpy as np
import ml_dtypes

import concourse.bass as bass
import concourse.bacc as bacc
import concourse.tile as tile
from concourse import mybir
from concourse.bass_utils import run_bass_kernel_spmd

B, S, D, H, DH = 2, 2048, 2048, 16, 128
NSC_G = 8  # global 512-col s-chunks over batch*seq
NCORES = 8
HL = H // NCORES          # 2 local heads per core
BS = B * S                # 4096
HD = HL * DH              # 256 local head dims per core
SCALE = 1.0 / math.sqrt(DH)
C0 = 14.0                 # bound for scale*raw_score (empirical max ~8.7)
NEG = -1.0e6              # raw-units additive causal mask (-8.8e4 after scale)

F32 = mybir.dt.float32
BF16 = mybir.dt.bfloat16
F16 = mybir.dt.float16

_SLOPES = [2.0 ** (-(i + 1) / 2.0) for i in range(H)]

# core c owns heads (c, c + 8): local slot lh=0 covers heads 0-7, lh=1
# covers heads 8-15. ALiBi decay lets the program skip k-tiles whose
# whole contribution is < e^-DROP_T relative; the skip set must be the
# union over cores, so it is governed by the smallest slope in each slot.
DROP_T = 8.0
_SLOT_MIN_SLOPE = [_SLOPES[7], _SLOPES[15]]


def _heads(c):
    return [c, c + 8]


def _kept_kts(lh, qc):
    kts = []
    for kt in range(4 * qc + 4):
        dist = qc * 512 - (kt * 128 + 127)
        if dist > 0 and _SLOT_MIN_SLOPE[lh] * dist > DROP_T:
            continue
        kts.append(kt)
    return kts


def _build_nc() -> bass.Bass:
    nc = bacc.Bacc("TRN2", target_bir_lowering=False, debug=False, num_devices=NCORES)

    xt_d = nc.dram_tensor("xt", [NSC_G, 128, 8192], BF16, kind="ExternalInput")
    wq_d = nc.dram_tensor("wq_t", [128, (D // 128) * HD], BF16, kind="ExternalInput")
    wk_d = nc.dram_tensor("wk_t", [128, (D // 128) * HD], BF16, kind="ExternalInput")
    wv_d = nc.dram_tensor("wv_t", [128, (D // 128) * HD], BF16, kind="ExternalInput")
    wo_d = nc.dram_tensor("wo_t", [128, HL * D], BF16, kind="ExternalInput")
    # mask: [:, :512] slot0 slope-mask (+causal), [:, 512:] causal-only
    mask_d = nc.dram_tensor("mask", [128, 2 * 512], F32, kind="ExternalInput")
    qrow_d = nc.dram_tensor("qrow", [128, 4 * 512], F32, kind="ExternalInput")
    # kbias: 0:16 slot0 j<0 per-kt, 16 slot0 diag, 17+qc*16+kt slot1
    kbias_d = nc.dram_tensor("kbias", [128, 17 + 64], F32, kind="ExternalInput")
    out_d = nc.dram_tensor("out_t", [D, BS], F16, kind="ExternalOutput")

    ND = D // 128   # 16 d-tiles
    NQC = S // 512  # 4 q-chunks per batch

    with tile.TileContext(nc) as tc, ExitStack() as ctx:
        const = ctx.enter_context(tc.tile_pool(name="const", bufs=1))
        xt_pool = ctx.enter_context(tc.tile_pool(name="xt", bufs=2))
        pt_pool = ctx.enter_context(tc.tile_pool(name="pt", bufs=6))
        pts_pool = ctx.enter_context(tc.tile_pool(name="pts", bufs=3))
        lred_pool = ctx.enter_context(tc.tile_pool(name="lred", bufs=2))
        rc_pool = ctx.enter_context(tc.tile_pool(name="rc", bufs=2))
        oe_pool = ctx.enter_context(tc.tile_pool(name="oe", bufs=8))

        # ---- resident constants / weights ----
        # wq in 4 quarter-pieces so the first QKV matmuls can start early
        wq_sb = [const.tile([128, 4 * HD], BF16, tag=f"wq{h}", name=f"wq{h}")
                 for h in range(4)]
        wk_sb = [const.tile([128, 4 * HD], BF16, tag=f"wk{h}", name=f"wk{h}")
                 for h in range(4)]
        wv_sb = [const.tile([128, 4 * HD], BF16, tag=f"wv{h}", name=f"wv{h}")
                 for h in range(4)]
        wo_sb = const.tile([128, HL * D], BF16, tag="wo")
        mask_sb = const.tile([128, 2 * 512], F32, tag="mask")
        qrow_sb = const.tile([128, 4 * 512], F32, tag="qrow")
        kbias_sb = const.tile([128, 17 + 64], F32, tag="kbias")
        ones_sb = const.tile([128, 128], BF16, tag="ones")

        # priority-ordered DMA queues: scalar carries wq (first need),
        # gpsimd the remaining weights; attention consts go on sync after
        # the first chunk's xt pieces (emitted in the schedule below).
        for h in range(4):
            nc.scalar.dma_start(
                out=wq_sb[h][:], in_=wq_d.ap()[:, h * 4 * HD:(h + 1) * 4 * HD]
            )
        for h in range(4):
            nc.gpsimd.dma_start(
                out=wk_sb[h][:], in_=wk_d.ap()[:, h * 4 * HD:(h + 1) * 4 * HD]
            )
        for h in range(4):
            nc.gpsimd.dma_start(
                out=wv_sb[h][:], in_=wv_d.ap()[:, h * 4 * HD:(h + 1) * 4 * HD]
            )
        nc.gpsimd.dma_start(out=wo_sb[:], in_=wo_d.ap())
        nc.vector.memset(ones_sb[:], 1.0)

        # ---- fine-grained resident activations ----
        qt_sb = [[[const.tile([128, 512], BF16, tag=f"qt{lh}{b}{qc}", name=f"qt{lh}{b}{qc}")
                   for qc in range(NQC)] for b in range(B)] for lh in range(HL)]
        kt_sb = [[[const.tile([128, 512], BF16, tag=f"kt{lh}{b}{qc}", name=f"kt{lh}{b}{qc}")
                   for qc in range(NQC)] for b in range(B)] for lh in range(HL)]
        v_sb = [[const.tile([128, HD], BF16, tag=f"v{b}_{st}", name=f"v{b}_{st}")
                 for st in range(16)] for b in range(B)]
        zt_sb = [[[const.tile([128, 512], BF16, tag=f"zt{lh}{b}{qc}", name=f"zt{lh}{b}{qc}")
                   for qc in range(NQC)] for b in range(B)] for lh in range(HL)]

        with ExitStack() as pctx:
            ps_mm = pctx.enter_context(tc.tile_pool(name="ps_mm", bufs=4, space="PSUM"))
            ps_z = pctx.enter_context(tc.tile_pool(name="ps_z", bufs=2, space="PSUM"))
            ps_l = pctx.enter_context(tc.tile_pool(name="ps_l", bufs=2, space="PSUM"))

            def qkv_chunk(b, scb, split_first=False):
                sc = b * NQC + scb
                if split_first:
                    # sc==0: stream xt in 4 dt-group pieces for fast start
                    xt_pieces = [
                        xt_pool.tile([128, 4 * 512], BF16, tag=f"xtp{p}", name=f"xtp{p}")
                        for p in range(4)
                    ]
                    for p, eng in ((0, nc.sync), (1, nc.sync), (2, nc.sync), (3, nc.sync)):
                        eng.dma_start(
                            out=xt_pieces[p][:],
                            in_=xt_d.ap()[sc, :, p * 2048:(p + 1) * 2048],
                        )


                    def xt_sl(dt, lo, size):
                        piece = xt_pieces[dt // 4]
                        base = (dt % 4) * 512 + lo
                        return piece[:, base:base + size]
                else:
                    xt_halves = [
                        xt_pool.tile([128, 8 * 512], BF16, tag=f"xt{h}", name=f"xt_{sc}_{h}")
                        for h in range(2)
                    ]
                    for h, eng in ((0, nc.sync), (1, nc.scalar)):
                        eng.dma_start(
                            out=xt_halves[h][:],
                            in_=xt_d.ap()[sc, :, h * 4096:(h + 1) * 4096],
                        )

                    def xt_sl(dt, lo, size):
                        half = xt_halves[dt // 8]
                        base = (dt % 8) * 512 + lo
                        return half[:, base:base + size]

                for w_src, dest in ((wq_sb, qt_sb), ((wk_sb,), kt_sb)):
                    for lh in range(HL):
                        psum = ps_mm.tile([128, 512], F32, tag="mm")
                        for dt in range(ND):
                            if w_src is wq_sb:
                                w_sl = wq_sb[dt // 4][:, (dt % 4) * HD + lh * 128:
                                                      (dt % 4) * HD + lh * 128 + 128]
                            else:
                                w_sl = wk_sb[dt // 4][:, (dt % 4) * HD + lh * 128:
                                                      (dt % 4) * HD + lh * 128 + 128]
                            nc.tensor.matmul(
                                psum[:], w_sl, xt_sl(dt, 0, 512),
                                start=(dt == 0), stop=(dt == ND - 1),
                            )
                        nc.vector.tensor_copy(dest[lh][b][scb][:], psum[:])
                for ss in range(4):
                    psum = ps_mm.tile([128, HD], F32, tag="mm")
                    for dt in range(ND):
                        nc.tensor.matmul(
                            psum[:],
                            xt_sl(dt, ss * 128, 128),
                            wv_sb[dt // 4][:, (dt % 4) * HD:(dt % 4) * HD + HD],
                            start=(dt == 0), stop=(dt == ND - 1),
                        )
                    nc.scalar.copy(v_sb[b][scb * 4 + ss][:], psum[:])

            def attn_chunk(b, qc):
                # small chunks: denominator via per-tile PE ones-matmuls
                # (cheap, keeps DVE free); big chunks: DVE-accumulate P and
                # one ones-matmul on the sum (saves PE streaming).
                l_on_pe = qc <= 1
                for lh in range(HL):
                    kts = _kept_kts(lh, qc)
                    zpsum = ps_z.tile([128, 512], F32, tag="z")
                    lpsum = ps_l.tile([128, 512], F32, tag="l")
                    ptsum = None
                    if not l_on_pe:
                        ptsum = pts_pool.tile([128, 512], F32, tag="pts")
                    pending = []
                    for i, kt in enumerate(kts):
                        j = kt - 4 * qc
                        lo = 128 * j if j >= 0 else 0
                        n = 512 - lo
                        spsum = ps_mm.tile([128, 512], F32, tag="mm")
                        nc.tensor.matmul(
                            spsum[:, 0:n],
                            kt_sb[lh][b][kt // 4][:, (kt % 4) * 128:(kt % 4) * 128 + 128],
                            qt_sb[lh][b][qc][:, lo:512],
                            start=True, stop=True,
                        )
                        if lh == 0:
                            if j >= 0:
                                add_sl = mask_sb[:, 0:n]
                                bias_sl = kbias_sb[:, 16:17]
                            else:
                                add_sl = qrow_sb[:, qc * 512:(qc + 1) * 512]
                                bias_sl = kbias_sb[:, kt:kt + 1]
                            nc.vector.tensor_add(spsum[:, 0:n], spsum[:, 0:n], add_sl)
                        else:
                            if j >= 0:
                                nc.vector.tensor_add(
                                    spsum[:, 0:n], spsum[:, 0:n],
                                    mask_sb[:, 512:512 + n],
                                )
                            bc = 17 + qc * 16 + kt
                            bias_sl = kbias_sb[:, bc:bc + 1]
                        pt = pt_pool.tile([128, 512], BF16, tag="pt")
                        nc.scalar.activation(
                            pt[:, 0:n], spsum[:, 0:n],
                            mybir.ActivationFunctionType.Exp,
                            bias=bias_sl,
                            scale=SCALE,
                        )
                        if not l_on_pe:
                            if i == 0:
                                nc.vector.tensor_copy(ptsum[:], pt[:])
                            else:
                                nc.vector.tensor_add(
                                    ptsum[:, lo:512], ptsum[:, lo:512], pt[:, 0:n])

                        def lz(i, kt, lo, n, pt):
                            if l_on_pe:
                                nc.tensor.matmul(
                                    lpsum[:, lo:512], ones_sb[:], pt[:, 0:n],
                                    start=(i == 0), stop=(i == len(kts) - 1),
                                    skip_group_check=True,
                                )
                            nc.tensor.matmul(
                                zpsum[:, lo:512],
                                v_sb[b][kt][:, lh * 128:(lh + 1) * 128],
                                pt[:, 0:n],
                                start=(i == 0), stop=(i == len(kts) - 1),
                                skip_group_check=True,
                            )

                        pending.append((i, kt, lo, n, pt))
                        if len(pending) > 1:
                            lz(*pending.pop(0))
                    for p in pending:
                        lz(*p)
                    if not l_on_pe:
                        ptsb = lred_pool.tile([128, 512], BF16, tag="ptsb")
                        nc.vector.tensor_copy(ptsb[:], ptsum[:])
                        nc.tensor.matmul(lpsum[:], ones_sb[:], ptsb[:],
                                         start=True, stop=True)
                    recip = rc_pool.tile([128, 512], F32, tag="rc")
                    nc.vector.reciprocal_approx_fast(recip[:], lpsum[:])
                    nc.vector.tensor_mul(zt_sb[lh][b][qc][:], zpsum[:], recip[:])

            n_out = [0]

            def outproj_sc(b, scb, tail=False):
                # all 16 o-tiles for s-chunk (b, scb); zt for both lh ready
                sc = b * NQC + scb
                for ot in range(16):
                    psum = ps_mm.tile([128, 512], F32, tag="mm")
                    for lh in range(HL):
                        nc.tensor.matmul(
                            psum[:],
                            wo_sb[:, lh * D + ot * 128: lh * D + ot * 128 + 128],
                            zt_sb[lh][b][scb][:],
                            start=(lh == 0), stop=(lh == HL - 1),
                        )
                    o_sb = oe_pool.tile([128, 512], F16, tag="oe")
                    if tail and ot % 2 == 1:
                        nc.vector.tensor_copy(o_sb[:], psum[:])
                    else:
                        nc.scalar.copy(o_sb[:], psum[:])
                    dma_eng = (nc.sync, nc.gpsimd, nc.sync, nc.scalar)[n_out[0] % 4]
                    dma_eng.dma_start(
                        out=out_d.ap()[ot * 128:(ot + 1) * 128, sc * 512:(sc + 1) * 512],
                        in_=o_sb[:],
                    )
                    n_out[0] += 1

            # ---- interleaved emission ----
            qkv_chunk(0, 0, split_first=True)
            nc.gpsimd.dma_start(out=mask_sb[:], in_=mask_d.ap())
            nc.gpsimd.dma_start(out=kbias_sb[:], in_=kbias_d.ap())
            nc.gpsimd.dma_start(out=qrow_sb[:], in_=qrow_d.ap())
            qkv_chunk(0, 1)
            attn_chunk(0, 0)
            qkv_chunk(0, 2)
            attn_chunk(0, 1)
            outproj_sc(0, 0)
            qkv_chunk(0, 3)
            attn_chunk(0, 2)
            outproj_sc(0, 1)
            qkv_chunk(1, 0)
            attn_chunk(0, 3)
            outproj_sc(0, 2)
            qkv_chunk(1, 1)
            attn_chunk(1, 0)
            outproj_sc(0, 3)
            qkv_chunk(1, 2)
            attn_chunk(1, 1)
            outproj_sc(1, 0)
            qkv_chunk(1, 3)
            attn_chunk(1, 2)
            outproj_sc(1, 1)
            attn_chunk(1, 3)
            outproj_sc(1, 2, tail=True)
            outproj_sc(1, 3, tail=True)

    nc.finalize()
    return nc


_NC = None


def _get_nc() -> bass.Bass:
    global _NC
    if _NC is None:
        _NC = _build_nc()
    return _NC


def _make_in_maps(resid_pre, Wq, Wk, Wv, Wo):
    bf = ml_dtypes.bfloat16
    x = np.asarray(resid_pre, np.float32).reshape(BS, D)
    # pre-tiled DMA-friendly layout: xt[sc, p, dt*512 + s] = x[sc*512+s, dt*128+p]
    xt = np.ascontiguousarray(
        x.reshape(NSC_G, 512, D // 128, 128).transpose(0, 3, 2, 1).reshape(NSC_G, 128, 8192)
    ).astype(bf)

    p = np.arange(128)[:, None]
    f = np.arange(512)[None, :]

    Wq = np.asarray(Wq, np.float32)
    Wk = np.asarray(Wk, np.float32)
    Wv = np.asarray(Wv, np.float32)
    Wo = np.asarray(Wo, np.float32)

    in_maps = []
    for c in range(NCORES):
        rows = np.r_[c * DH:(c + 1) * DH, (c + 8) * DH:(c + 9) * DH]
        s0 = _SLOPES[c]       # slot0 slope (exact per-q shift convention)
        s1 = _SLOPES[c + 8]   # slot1 slope (per-chunk constant shift)
        qrow = np.zeros((128, 4 * 512), np.float32)
        kbias = np.zeros((128, 17 + 64), np.float32)
        mask = np.zeros((128, 2 * 512), np.float32)
        for qc in range(4):
            q = qc * 512 + np.arange(512, dtype=np.float64)
            qrow[:, qc * 512:(qc + 1) * 512] = (-s0 * q / SCALE)[None, :].astype(np.float32)
        for kt in range(16):
            kbias[:, kt] = (
                s0 * (kt * 128 + np.arange(128, dtype=np.float64)) - C0
            ).astype(np.float32)
        kbias[:, 16] = (s0 * np.arange(128, dtype=np.float64) - C0).astype(np.float32)
        # slot0 diagonal-band mask: exp arg = scale*s + s0*(p - q'') - C0
        mask[:, 0:512] = ((-s0 * f / SCALE) + np.where(p > f, NEG, 0.0)).astype(np.float32)
        # slot1: causal-only mask; alibi fully in per-(qc,kt) bias with
        # chunk-end shift: arg = scale*s + s1*(k - (qc*512+511)) - C0
        mask[:, 512:1024] = np.where(p > f, NEG, 0.0).astype(np.float32)
        for qc in range(4):
            for kt in range(4 * qc + 4):
                kbias[:, 17 + qc * 16 + kt] = (
                    s1 * (kt * 128 + np.arange(128, dtype=np.float64)
                          - (qc * 512 + 511)) - C0
                ).astype(np.float32)
        in_maps.append({
            "xt": xt,
            # [p, dt*HD + m] = W.T[dt*128+p, m]  (contiguous 8KB rows)
            "wq_t": np.ascontiguousarray(
                Wq[rows, :].T.reshape(D // 128, 128, HD).transpose(1, 0, 2).reshape(128, -1)
            ).astype(bf),
            "wk_t": np.ascontiguousarray(
                Wk[rows, :].T.reshape(D // 128, 128, HD).transpose(1, 0, 2).reshape(128, -1)
            ).astype(bf),
            "wv_t": np.ascontiguousarray(
                Wv[rows, :].T.reshape(D // 128, 128, HD).transpose(1, 0, 2).reshape(128, -1)
            ).astype(bf),
            # [p, lh*D + o] = Wo[:, rows].T[lh*128+p, o]
            "wo_t": np.ascontiguousarray(
                Wo[:, rows].T.reshape(HL, 128, D).transpose(1, 0, 2).reshape(128, -1)
            ).astype(bf),
            "mask": mask,
            "qrow": qrow,
            "kbias": kbias,
        })
    return in_maps


def _combine(results) -> np.ndarray:
    acc = np.zeros((D, BS), np.float32)
    for m in results:
        acc += m["out_t"].astype(np.float32)
    return np.ascontiguousarray(acc.reshape(D, B, S).transpose(1, 2, 0))


def kernel(resid_pre, Wq, Wk, Wv, Wo):
    nc = _get_nc()
    in_maps = _make_in_maps(resid_pre, Wq, Wk, Wv, Wo)
    res = run_bass_kernel_spmd(nc, in_maps, core_ids=list(range(NCORES)))
    return _combine(res.results)

